# revision 25
# baseline (speedup 1.0000x reference)
"""Trainium2 Bass kernel for nn_Attention_386547057357 (Transformer-XL style
relative-position sparse attention).

Sharding: data-parallel over batch - core c computes batch element c.
All weights replicated per core.

Math (per batch element):
    X = [memory; x]  (1024, 512)
    q = x @ (W_q * scale)  (256, 512);  k = X @ W_k;  v = X @ W_v
    qhat = q + u_emb*scale (per head);  qtld = q + v_emb*scale
    RW = R @ W_rel  (1024, 512)         [host precomputed constant]
    ac[n, m]  = qhat_h[n] . k_h[m]              (= (term_a + term_c) * scale)
    bd[n, r'] = qtld_h[n] . RW[1023 - r']       (= (term_b + term_d) * scale, reversed r)
    scores[n, m] = ac[n, m] + bd[n, 255 - n + m]  + causal mask
    out = softmax(scores) @ v @ W_out + b_out

Implementation notes:
  - the relative-position shift is an SBUF->SBUF DMA with a diagonal access
    pattern: bd rows live in a [128, 1280] tile (cols [1024:1280) = -1e30 pad),
    and the shifted read uses AP [[1279, 128], [1, 1024]] at offset 255-128*n2,
    so row p reads cols [off-p, off-p+1024) - a per-partition shift.  The pad
    lands exactly on the causally-masked region, so the mask is free.
  - shifted bd is merged into the ac PSUM with an identity matmul (PE
    accumulate), then a single ACT exp (f32 psum -> bf16) per tile.
  - softmax denominator comes free from a ones-column appended to V (row 64 of
    the PV PSUM); normalization happens after PV on the small [64, 256] output
    (reciprocal + PE ones-broadcast + one small DVE multiply).
  - RW^T = (R @ W_rel) is precomputed on the host (it is just a reparametrized
    weight); this removes all fp32 matmuls from the device.
  - softmax without max subtraction (logit magnitudes are small; fp32 exp).
  - attn transpose for the PV matmul via SBUF-source dma_gather(transpose=True).
  - all PE operands bf16; PSUM accumulation fp32; output fp32.
"""

import sys

sys.path.insert(0, "/opt/trn_rl_repo")

import numpy as np
import ml_dtypes

import concourse.bass as bass
import concourse.mybir as mybir
import concourse.tile as tile
from concourse import bacc, library_config
from concourse.bass_utils import run_bass_kernel_spmd
from concourse.tile_rust import add_dep_helper

BF16 = ml_dtypes.bfloat16
F32 = np.float32

DIM = 512
NHEAD = 8
DHEAD = 64
CTX = 1024
NOCT = 11
B = 8
SEQ = 256
MEM = 768
TOT = MEM + SEQ  # 1024
SCALE = DHEAD ** -0.5  # 0.125
BDW = 1280  # bd scratch row width (1024 data + 256 pad)
NEG = -1e30

dt = mybir.dt
AF = mybir.ActivationFunctionType
ALU = mybir.AluOpType


# ---------------------------------------------------------------- host consts
def _positional_encoding():
    coords = np.arange(CTX, dtype=F32)[:, None]
    octaves = np.arange(1 - NOCT, 1, dtype=F32)
    mult = ((2.0 ** octaves) * np.pi).astype(F32)
    scaled = (coords * mult[None, :]).astype(F32)
    return np.concatenate([np.sin(scaled), np.cos(scaled)], axis=-1).astype(F32)


def _chunked(w, nchunk):
    """(128*nchunk, F) -> (128, nchunk, F) with [p, c, f] = w[128c + p, f]."""
    f = w.shape[1]
    return np.ascontiguousarray(w.reshape(nchunk, 128, f).transpose(1, 0, 2))


# ---------------------------------------------------------------- bass program
def build_program():
    nc = bacc.Bacc("TRN2", target_bir_lowering=False, debug=False)

    xt_d = nc.dram_tensor("xt", [128, 4, TOT], dt.bfloat16, kind="ExternalInput")
    wqkv_d = nc.dram_tensor("wqkv", [128, 4, 1536], dt.bfloat16, kind="ExternalInput")
    rwt_d = nc.dram_tensor("rwt", [128, 4, CTX], dt.bfloat16, kind="ExternalInput")
    wouth_d = nc.dram_tensor("wouth", [64, 8, 512], dt.bfloat16, kind="ExternalInput")
    bout_d = nc.dram_tensor("bout", [128, 512], dt.float32, kind="ExternalInput")
    u2s_d = nc.dram_tensor("u2s", [128, 1], dt.float32, kind="ExternalInput")
    v2s_d = nc.dram_tensor("v2s", [128, 1], dt.float32, kind="ExternalInput")
    ident_d = nc.dram_tensor("ident", [128, 128], dt.bfloat16, kind="ExternalInput")
    gidx_d = nc.dram_tensor("gidx", [128, 16], dt.int16, kind="ExternalInput")
    out_d = nc.dram_tensor("out", [SEQ, 512], dt.float32, kind="ExternalOutput")

    with tile.TileContext(nc) as tc:
        _body(tc, xt_d, wqkv_d, rwt_d, wouth_d, bout_d, u2s_d, v2s_d, ident_d,
              gidx_d, out_d)
    nc.compile()
    return nc


def _body(tc, xt_d, wqkv_d, rwt_d, wouth_d, bout_d, u2s_d, v2s_d, ident_d,
          gidx_d, out_d):
    nc = tc.nc
    from contextlib import ExitStack

    with ExitStack() as ctx:
        consts = ctx.enter_context(tc.tile_pool(name="consts", bufs=1))

        # ---- load constants / weights.  Queues: sync = tiny consts + xt +
        # wouth + bout, scalar(ACT-HWDGE) = wqkv + rwt.  Tiny consts go FIRST
        # (u2s/v2s gate the qT psum eviction and hence the mps pool rotation).
        u2s = consts.tile([128, 1], dt.float32)
        nc.gpsimd.dma_start(u2s[:], u2s_d.ap())
        v2s = consts.tile([128, 1], dt.float32)
        nc.gpsimd.dma_start(v2s[:], v2s_d.ap())
        gidx = consts.tile([128, 16], dt.int16)
        nc.gpsimd.dma_start(gidx[:], gidx_d.ap())
        ident = consts.tile([128, 128], dt.bfloat16)
        nc.gpsimd.dma_start(ident[:], ident_d.ap())
        xt = consts.tile([128, 4, TOT], dt.bfloat16)
        nc.sync.dma_start(xt[:, :, MEM:TOT], xt_d.ap()[:, :, MEM:TOT])
        nc.sync.dma_start(xt[:, :, 512:MEM], xt_d.ap()[:, :, 512:MEM])
        nc.sync.dma_start(xt[:, :, 0:512], xt_d.ap()[:, :, 0:512])
        wouth = consts.tile([64, 8, 512], dt.bfloat16)
        nc.sync.dma_start(wouth[:], wouth_d.ap())
        bout = consts.tile([128, 512], dt.float32)
        nc.sync.dma_start(bout[:], bout_d.ap())

        wqkv = consts.tile([128, 4, 1536], dt.bfloat16)
        nc.scalar.dma_start(wqkv[:, :, 0:512], wqkv_d.ap()[:, :, 0:512])
        nc.scalar.dma_start(wqkv[:, :, 512:768], wqkv_d.ap()[:, :, 512:768])
        nc.scalar.dma_start(wqkv[:, :, 768:1024], wqkv_d.ap()[:, :, 768:1024])
        rwt = consts.tile([128, 4, CTX], dt.bfloat16)
        nc.scalar.dma_start(rwt[:, 0:2, :], rwt_d.ap()[:, 0:2, :])
        nc.scalar.dma_start(wqkv[:, :, 1024:1536], wqkv_d.ap()[:, :, 1024:1536])
        nc.scalar.dma_start(rwt[:, 2:4, :], rwt_d.ap()[:, 2:4, :])

        lib_inst = nc.gpsimd.load_library(library_config.mlp)

        # persistent intermediates
        qhatT = consts.tile([128, 4, SEQ], dt.bfloat16)  # (q+u)^T  [hd, n]
        qtldT = consts.tile([128, 4, SEQ], dt.bfloat16)  # (q+v)^T  [hd, n]
        kT = consts.tile([128, 4, TOT], dt.bfloat16)     # k^T      [hd, m]
        vvaug = consts.tile([128, 8, 512], dt.bfloat16)  # V        [m, hd]
        avt64 = consts.tile([64, 8, SEQ], dt.bfloat16)   # attnV^T  [d, h, n]
        ones128 = consts.tile([128, 128], dt.bfloat16)   # rows 64:66 for bcast
        nc.vector.memset(ones128[:], 1.0)
        recT2sb = consts.tile([128, SEQ], dt.bfloat16)   # row 64: recT
        bdslots = []
        for s4 in range(4):
            t = consts.tile([128, BDW], dt.bfloat16, name=f"bdslot{s4}")
            nc.vector.memset(t[:, TOT:BDW], NEG)         # causal-mask pad, once
            bdslots.append(t)

        with (
            tc.tile_pool(name="mps", bufs=3, space="PSUM") as mps,
            tc.tile_pool(name="pvps", bufs=2, space="PSUM") as pvps,
            tc.tile_pool(name="hsb", bufs=4) as hsb,
            tc.tile_pool(name="shp", bufs=4) as shp,
            tc.tile_pool(name="aup", bufs=3) as aup,
            tc.tile_pool(name="atp", bufs=4) as atp,
            tc.tile_pool(name="dnp", bufs=8) as dnp,
        ):
            # ---- PE warm-up: the HAM clock gate keeps an idle PE at 1.2GHz
            # (K=4/8) and only releases to 2.4GHz after ~3.4us of sustained
            # activity.  Issue dummy matmuls during the initial weight-DMA
            # wait so the real encode phase starts at full clock.
            wtile = hsb.tile([128, 128], dt.bfloat16, tag="warm")
            nc.vector.memset(wtile[:], 0.0)
            wps = mps.tile([128, 1024], dt.float32, tag="m")
            for i in range(72):
                nc.tensor.matmul(wps[:, 0:128], wtile[:], wtile[:],
                                 start=True, stop=True)

            # ---- q^T per head pair: q = x @ Wq_scaled; qhat/qtld = q + u/v
            for hp in range(4):
                ps = mps.tile([128, 1024], dt.float32, tag="m")
                for ch in range(4):
                    nc.tensor.matmul(ps[:, 0:SEQ],
                                     wqkv[:, ch, 128 * hp:128 * (hp + 1)],
                                     xt[:, ch, MEM:TOT],
                                     start=(ch == 0), stop=(ch == 3))
                qf = hsb.tile([128, SEQ], dt.float32, tag="qf")
                nc.vector.tensor_scalar_add(qf[:], ps[:, 0:SEQ], u2s[:])
                nc.vector.tensor_copy(qhatT[:, hp, :], qf[:])
                qf2 = hsb.tile([128, SEQ], dt.float32, tag="qf")
                nc.vector.tensor_scalar_add(qf2[:], ps[:, 0:SEQ], v2s[:])
                nc.vector.tensor_copy(qtldT[:, hp, :], qf2[:])

            # ---- HAM keep-alive while wqkv_k lands (prevents the PE clock
            # gate from re-throttling during the input wait)
            wps2 = mps.tile([128, 1024], dt.float32, tag="m")
            for i in range(24):
                nc.tensor.matmul(wps2[:, 0:128], wtile[:], wtile[:],
                                 start=True, stop=True)

            # ---- k^T per head pair
            for hp in range(4):
                ps = mps.tile([128, 1024], dt.float32, tag="m")
                for mh in range(2):
                    for ch in range(4):
                        nc.tensor.matmul(
                            ps[:, 512 * mh:512 * (mh + 1)],
                            wqkv[:, ch, 512 + 128 * hp:512 + 128 * (hp + 1)],
                            xt[:, ch, 512 * mh:512 * (mh + 1)],
                            start=(ch == 0), stop=(ch == 3))
                nc.vector.tensor_copy(kT[:, hp, :], ps[:])

            # ---- V in augmented [m, (hd|ones) x 8] layout
            def emit_v():
                for mc0 in (0, 2, 4, 6):
                    ps = mps.tile([128, 1024], dt.float32, tag="m")
                    for k2 in range(2):
                        mc = mc0 + k2
                        for ch in range(4):
                            nc.tensor.matmul(
                                ps[:, 512 * k2:512 * (k2 + 1)],
                                xt[:, ch, 128 * mc:128 * (mc + 1)],
                                wqkv[:, ch, 1024:1536],
                                start=(ch == 0), stop=(ch == 3))
                    nc.vector.tensor_copy(vvaug[:, mc0:mc0 + 2, :], ps[:])

            # ---- bd matmul -> bf16 padded slot -> diagonal-shift DMA
            shs = {}
            bd_count = [0]

            def emit_bd(h, n2):
                hp, pb = h // 2, 64 * (h % 2)
                ps = mps.tile([128, 1024], dt.float32, tag="m")
                for rh in range(2):
                    nc.tensor.matmul(
                        ps[:, 512 * rh:512 * (rh + 1)],
                        qtldT[pb:pb + 64, hp, 128 * n2:128 * (n2 + 1)],
                        rwt[pb:pb + 64, hp, 512 * rh:512 * (rh + 1)],
                        start=True, stop=True)
                slot = bdslots[bd_count[0] % 4]
                bd_count[0] += 1
                nc.vector.tensor_copy(slot[:, 0:TOT], ps[:])
                sh = shp.tile([128, TOT], dt.bfloat16, tag="sh")
                # row p reads slot cols [off - p, off - p + TOT) -- the shift
                src = bass.AP(slot.tensor, slot.offset + (255 - 128 * n2),
                              [[BDW - 1, 128], [1, TOT]])
                nc.sync.dma_start(sh[:], src)
                shs[(h, n2)] = sh

            # ---- ac matmul + PE merge of shifted bd + fused exp->bf16
            attnUs = {}
            dens = {}

            def emit_ac(h, n2):
                hp, pb = h // 2, 64 * (h % 2)
                if n2 == 0:
                    attnUs[h] = aup.tile([128, 2, TOT], dt.bfloat16, tag="au", name=f"au{h}")
                ps = mps.tile([128, 1024], dt.float32, tag="m")
                for mh in range(2):
                    nc.tensor.matmul(
                        ps[:, 512 * mh:512 * (mh + 1)],
                        qhatT[pb:pb + 64, hp, 128 * n2:128 * (n2 + 1)],
                        kT[pb:pb + 64, hp, 512 * mh:512 * (mh + 1)],
                        start=True, stop=False)
                sh = shs.pop((h, n2))
                for mh in range(2):
                    nc.tensor.matmul(
                        ps[:, 512 * mh:512 * (mh + 1)],
                        ident[:],
                        sh[:, 512 * mh:512 * (mh + 1)],
                        start=False, stop=True)
                den = dnp.tile([128, 1], dt.float32, tag="den")
                dens[(h, n2)] = den
                nc.scalar.activation(attnUs[h][:, n2, :], ps[:],
                                     func=AF.Exp, scale=1.0, accum_out=den[:])

            # ---- on-chip transpose (gpsimd gather)
            attnTs = {}

            def emit_gather(h):
                attnT = atp.tile([128, 8, SEQ], dt.bfloat16, tag="at")
                attnTs[h] = attnT
                g = nc.gpsimd.dma_gather(
                    out_ap=attnT[:], in_ap=attnUs[h][:], idxs_ap=gidx[:],
                    num_idxs=SEQ, num_idxs_reg=SEQ, elem_size=TOT,
                    transpose=True, sbuf_tokens_per_rank=128,
                    sbuf_free_dim_per_rank=2 * TOT,
                    sbuf_free_dim_pad_per_rank=0, sbuf_byte_offset=0)
                add_dep_helper(g.ins, lib_inst.ins,
                               reason="dma_gather needs mlp gpsimd library")

            # ---- PV + normalize after PV.  The softmax denominator comes
            # from the exp's accum_out (n-major [128,1], cheap reciprocal);
            # a tiny PE transpose (T1) + K=2 ones-broadcast matmul (T2) turn
            # 1/den into the hd-major [64,256] multiplier.  ([1,256]-shaped
            # DVE ops are one lane doing 256 elements -- avoided throughout.)
            def emit_pv(h):
                pv = pvps.tile([128, SEQ], dt.float32, tag="pv")
                rec2 = hsb.tile([128, 2], dt.float32, tag="rec2")
                nc.vector.reciprocal(rec2[:, 0:1], dens.pop((h, 0))[:])
                nc.vector.reciprocal(rec2[:, 1:2], dens.pop((h, 1))[:])
                recb2 = hsb.tile([128, 2], dt.bfloat16, tag="recb2")
                nc.vector.tensor_copy(recb2[:], rec2[:])
                # T1: recT[0, 128*n2 + j] = rec2[j, n2]  (row 64 of the pv psum)
                nc.tensor.matmul(pv[64:65, 0:128], recb2[:, 0:1],
                                 ident[:, 0:128], start=True, stop=True)
                nc.tensor.matmul(pv[64:65, 128:SEQ], recb2[:, 1:2],
                                 ident[:, 0:128], start=True, stop=True)
                nc.vector.tensor_copy(recT2sb[64:65, :], pv[64:65, :])
                attnT = attnTs.pop(h)
                for mc in range(8):
                    nc.tensor.matmul(pv[0:64, :],
                                     vvaug[:, mc, 64 * h:64 * h + 64],
                                     attnT[:, mc, :],
                                     start=(mc == 0), stop=(mc == 7))
                # T2: rep[d, n] = recT2sb[96] | recT2sb[97]  (K=2 broadcast)
                rep = pvps.tile([128, SEQ], dt.float32, tag="pv")
                nc.tensor.matmul(rep[0:64, :], ones128[64:65, 0:64],
                                 recT2sb[64:65, :], start=True, stop=True)
                repsb = hsb.tile([64, SEQ], dt.float32, tag="repsb")
                nc.vector.tensor_copy(repsb[:], rep[0:64, :])
                avtf = hsb.tile([64, SEQ], dt.float32, tag="avtf")
                nc.vector.tensor_mul(avtf[:], pv[0:64, :], repsb[:])
                nc.vector.tensor_copy(avt64[:, h, :], avtf[:])

            # ---- staggered pipeline over heads
            emit_bd(0, 0); emit_bd(0, 1)
            emit_v()
            for h in range(1, NHEAD):
                emit_bd(h, 0); emit_bd(h, 1)
                if 3 <= h <= 5:
                    # HAM keep-alive over the pipeline-fill dip
                    wpsk = mps.tile([128, 1024], dt.float32, tag="m")
                    for i in range(5):
                        nc.tensor.matmul(wpsk[:, 0:128], wtile[:], wtile[:],
                                         start=True, stop=True)
                emit_ac(h - 1, 0); emit_ac(h - 1, 1)
                emit_gather(h - 1)
                if h >= 3:
                    emit_pv(h - 3)
            emit_ac(7, 0); emit_ac(7, 1)
            emit_gather(7)
            emit_pv(5); emit_pv(6)
            wps3 = mps.tile([128, 1024], dt.float32, tag="m")
            for i in range(10):
                nc.tensor.matmul(wps3[:, 0:128], wtile[:], wtile[:],
                                 start=True, stop=True)
            emit_pv(7)

            # ---- output projection
            for n2 in range(2):
                psw = mps.tile([128, 1024], dt.float32, tag="m")
                ps = psw[:, 0:512]
                for h in range(8):
                    nc.tensor.matmul(ps,
                                     avt64[:, h, 128 * n2:128 * (n2 + 1)],
                                     wouth[:, h, :],
                                     start=(h == 0), stop=(h == 7))
                osb = hsb.tile([128, 512], dt.float32, tag="osb")
                nc.vector.tensor_add(osb[:], ps, bout[:])
                nc.sync.dma_start(out_d.ap()[128 * n2:128 * (n2 + 1), :], osb[:])


# ---------------------------------------------------------------- host wrapper
_PROGRAM = None


def _get_program():
    global _PROGRAM
    if _PROGRAM is None:
        _PROGRAM = build_program()
    return _PROGRAM


def make_in_maps(x, memory, W_qkv, W_rel, W_out, b_out, u_emb, v_emb):
    x = np.asarray(x, dtype=F32)
    memory = np.asarray(memory, dtype=F32)
    W_qkv = np.asarray(W_qkv, dtype=F32)
    W_rel = np.asarray(W_rel, dtype=F32)
    W_out = np.asarray(W_out, dtype=F32)
    b_out = np.asarray(b_out, dtype=F32)
    u_emb = np.asarray(u_emb, dtype=F32)
    v_emb = np.asarray(v_emb, dtype=F32)

    R = _positional_encoding()                       # (1024, 22)
    RW = (R @ W_rel).astype(F32)                     # (1024, 512)
    rwt = _chunked(np.ascontiguousarray(RW[::-1].T), 4).astype(BF16)

    W_qkv_s = W_qkv.copy()
    W_qkv_s[:, 0:512] *= SCALE                       # fold scale into W_q
    wqkv = _chunked(W_qkv_s, 4).astype(BF16)         # (128, 4, 1536)
    wouth = np.ascontiguousarray(
        W_out.reshape(8, 64, 512).transpose(1, 0, 2)).astype(BF16)
    bout = np.tile(b_out[None, :], (128, 1)).astype(F32)
    u2s = (np.tile(u_emb, 2)[:, None] * SCALE).astype(F32)
    v2s = (np.tile(v_emb, 2)[:, None] * SCALE).astype(F32)
    ident = np.eye(128, dtype=F32).astype(BF16)
    p = np.arange(128)[:, None] % 16
    s = np.arange(16)[None, :]
    gidx = (s * 16 + p).astype(np.int16)             # (128, 16)

    shared = dict(wqkv=wqkv, rwt=rwt, wouth=wouth, bout=bout,
                  u2s=u2s, v2s=v2s, ident=ident, gidx=gidx)
    in_maps = []
    for c in range(B):
        X = np.concatenate([memory[c], x[c]], axis=0)          # (1024, 512)
        xt = _chunked(np.ascontiguousarray(X.T), 4).astype(BF16)  # (128,4,1024)
        in_maps.append(dict(xt=xt, **shared))
    return in_maps


def run(in_maps, trace=False, **kw):
    nc = _get_program()
    res = run_bass_kernel_spmd(nc, in_maps, core_ids=list(range(B)),
                               trace=trace, **kw)
    out = np.stack([res.results[c]["out"] for c in range(B)]).astype(F32)
    return out, res


def kernel(x, memory, W_qkv, W_rel, W_out, b_out, u_emb, v_emb):
    in_maps = make_in_maps(x, memory, W_qkv, W_rel, W_out, b_out, u_emb, v_emb)
    out, _ = run(in_maps)
    return out.reshape(B, SEQ, DIM)
